# revision 1
# baseline (speedup 1.0000x reference)
"""FESTGCN Trainium2 kernel: 8-core SPMD Bass/Tile implementation.

Algorithm (validated against the reference in f32 numpy):
  For t in 0..9:
    M_t = dtw * (time_delay > 9-t) + (spec_lap + I)   [+ laplacian at t=9]
    S1 += M_t @ c1_t            c1_t = [x_t | h_t]            (x2/3 at t=9)
    gcn1_t = 0.5 * S1 @ W1 + (t+1) b1 ;  sig_t = sigmoid(gcn1_t)
    r_t = first flat half of sig_t (nodes < N/2, all 2H ch)
    S2 += M_t @ c2_t            c2_t = [x_t | r_t*h_t]        (x2/3 at t=9)
  u = second flat half of sig_9 ; c = tanh(0.5 * S2 @ W2 + 10 b2)
  out = u*h_9 + (1-u)*c

Sharding: node rows split across 8 cores (512 rows each). Each core holds the
transposed row-block of the NxN matrices ([n, m] layout, n on partitions, so
they feed the PE as stationary operands directly). Per step, each core
computes gcn1 for its own rows, all-gathers the sigmoid values (needed by
every core due to the flat-split gating), then runs the gated conv. The final
output is computed redundantly on every core from gathered data.
"""

import numpy as np

import concourse.bacc as bacc
import concourse.mybir as mybir
import concourse.tile as tile
from concourse.bass_utils import run_bass_kernel_spmd

B, T, N, H = 4, 10, 4096, 32
NC = 8
RPC = N // NC            # 512 rows per core
NT = N // 128            # 32 n-tiles
MT = RPC // 128          # 4 m-tiles per core
F1 = B * (H + 1)         # 132 moving columns per conv
NH = N * H               # 131072
SH = RPC * 2 * H         # 32768: per-rank AG shard elements per batch
f32 = mybir.dt.float32
bf16 = mybir.dt.float16  # fp16 data path: exact products, 8x less quant noise
Alu = mybir.AluOpType
Act = mybir.ActivationFunctionType
CORES = list(range(NC))


def _build_nc(dbg=False):
    nc = bacc.Bacc(
        "TRN2",
        target_bir_lowering=False,
        debug=False,
        enable_asserts=True,
        num_devices=NC,
    )
    # Per-core inputs. Big matrices arrive pre-transposed: [n, m_block].
    dtwT = nc.dram_tensor("dtwT", [N, RPC], f32, kind="ExternalInput").ap()
    tdT = nc.dram_tensor("tdT", [N, RPC], f32, kind="ExternalInput").ap()
    sleT = nc.dram_tensor("sleT", [N, RPC], f32, kind="ExternalInput").ap()
    lapT = nc.dram_tensor("lapT", [N, RPC], f32, kind="ExternalInput").ap()
    states = nc.dram_tensor("states", [T, B, NH], f32, kind="ExternalInput").ap()
    xT = nc.dram_tensor("xT", [T, N, B], f32, kind="ExternalInput").ap()
    w1h = nc.dram_tensor("w1h", [H + 1, 2 * H], f32, kind="ExternalInput").ap()
    w2h = nc.dram_tensor("w2h", [H + 1, H], f32, kind="ExternalInput").ap()
    biastab = nc.dram_tensor("biastab", [11, 256], f32, kind="ExternalInput").ap()
    hout = nc.dram_tensor("hout", [B, NH], f32, kind="ExternalOutput").ap()
    if dbg:
        dbg_td = nc.dram_tensor("dbg_td", [128, RPC], bf16, kind="ExternalOutput").ap()
        dbg_m = nc.dram_tensor("dbg_m", [128, RPC], bf16, kind="ExternalOutput").ap()
        dbg_z1 = nc.dram_tensor("dbg_z1", [128, 2 * F1], f32, kind="ExternalOutput").ap()
        dbg_sig = nc.dram_tensor("dbg_sig", [128, 256], bf16, kind="ExternalOutput").ap()
        dbg_ag = nc.dram_tensor("dbg_ag", [NC, B, SH], bf16, kind="ExternalOutput").ap()
        dbg_c2 = nc.dram_tensor("dbg_c2", [128, F1], bf16, kind="ExternalOutput").ap()
        dbg_z2 = nc.dram_tensor("dbg_z2", [128, 2 * F1], f32, kind="ExternalOutput").ap()
        dbg_g1 = nc.dram_tensor("dbg_g1", [128, 256], f32, kind="ExternalOutput").ap()

    with tile.TileContext(nc) as tc:
        with (
            tc.tile_pool(name="big", bufs=1) as big,        # resident bf16 matrices
            tc.tile_pool(name="stage", bufs=2) as stage,    # f32 staging
            tc.tile_pool(name="mpool", bufs=48) as mpool,   # masked+combined M tiles
            tc.tile_pool(name="mtmp", bufs=6) as mtmp,
            tc.tile_pool(name="c1p", bufs=8) as c1p,
            tc.tile_pool(name="c2p", bufs=6) as c2p,
            tc.tile_pool(name="rp", bufs=6) as rp,
            tc.tile_pool(name="sm", bufs=1) as sm,          # small persistents
            tc.tile_pool(name="acc", bufs=2) as accp,       # S1/S2 ping-pong
            tc.tile_pool(name="wk", bufs=6) as wk,          # small work tiles
            tc.tile_pool(name="z1p", bufs=1, space="PSUM") as z1p,
            tc.tile_pool(name="z2p", bufs=1, space="PSUM") as z2p,
            tc.tile_pool(name="tpz", bufs=2, space="PSUM") as tpzp,
            tc.tile_pool(name="g1p", bufs=2, space="PSUM") as g1p,
            tc.tile_pool(name="dramp", bufs=1, space="DRAM") as dramp,
        ):
            # AG buffers (one pair per step -> no WAR hazards).
            agsrc = [
                dramp.tile([B, SH], bf16, tag=f"agsrc{t}", name=f"agsrc{t}")
                for t in range(T)
            ]
            agdst = [
                dramp.tile([NC, B, SH], bf16, tag=f"agdst{t}", name=f"agdst{t}",
                           addr_space="Shared")
                for t in range(T)
            ]
            auxsrc = dramp.tile([B, RPC * H], bf16, tag="auxsrc", name="auxsrc")
            auxdst = dramp.tile([NC, B, RPC * H], bf16, tag="auxdst",
                                name="auxdst", addr_space="Shared")
            # ---------------- prologue ----------------
            # identity for PE transposes
            iota_i = wk.tile([128, 128], mybir.dt.int32, tag="iota", bufs=1)
            nc.gpsimd.iota(iota_i[:], pattern=[[1, 128]], base=0, channel_multiplier=-1)
            ident = sm.tile([128, 128], f32, tag="ident")
            nc.vector.tensor_scalar(ident[:], iota_i[:], 0, None, op0=Alu.is_equal)

            # weights / bias
            w1s = sm.tile([H + 1, 2 * H], f32, tag="w1s")
            nc.sync.dma_start(w1s[:], w1h[:])
            w2s = sm.tile([H + 1, H], f32, tag="w2s")
            nc.sync.dma_start(w2s[:], w2h[:])

            # resident bf16 matrices, tiled [128, RPC] over 32 n-tiles
            dtw_bf, td_bf, sle_bf = [], [], []
            for nt in range(NT):
                sl0 = nt * 128
                st_d = stage.tile([128, RPC], f32, tag="stg")
                nc.sync.dma_start(st_d[:], dtwT[sl0 : sl0 + 128, :])
                tb = big.tile([128, RPC], bf16, tag=f"dtw{nt}")
                nc.vector.tensor_copy(tb[:], st_d[:])
                dtw_bf.append(tb)

                st_s = stage.tile([128, RPC], f32, tag="stg")
                nc.sync.dma_start(st_s[:], sleT[sl0 : sl0 + 128, :])
                sb = big.tile([128, RPC], bf16, tag=f"sle{nt}")
                nc.vector.tensor_copy(sb[:], st_s[:])
                sle_bf.append(sb)

                # ceil(time_delay) as exact small ints in bf16, via the
                # round-to-nearest magic constant: round(v + 0.5) == ceil(v)
                # for non-integer v (exact-integer v is measure-zero here).
                st_t = stage.tile([128, RPC], f32, tag="stg")
                nc.sync.dma_start(st_t[:], tdT[sl0 : sl0 + 128, :])
                rmag = stage.tile([128, RPC], f32, tag="frs", bufs=2)
                nc.vector.tensor_scalar(
                    rmag[:], st_t[:], 0.5, 12582912.0, op0=Alu.add, op1=Alu.add
                )
                ctb = big.tile([128, RPC], bf16, tag=f"td{nt}")
                nc.vector.tensor_scalar(
                    ctb[:], rmag[:], 12582912.0, None, op0=Alu.subtract
                )
                td_bf.append(ctb)

            # x preload: [nt][128, (t, b)] bf16
            xall = []
            for nt in range(NT):
                xs = stage.tile([128, T * B], f32, tag="xst", bufs=2)
                nc.sync.dma_start(
                    xs.rearrange("p (t b) -> p t b", b=B),
                    xT[:, nt * 128 : (nt + 1) * 128, :].rearrange("t p b -> p t b"),
                )
                xb = sm.tile([128, T * B], bf16, tag=f"x{nt}")
                nc.vector.tensor_copy(xb[:], xs[:])
                xall.append(xb)

            # S1/S2 accumulators (f32) - ping-pong via pool bufs=2
            s1 = [accp.tile([128, F1], f32, tag=f"s1_{mt}", name=f"s1_{mt}") for mt in range(MT)]
            s2 = [accp.tile([128, F1], f32, tag=f"s2_{mt}", name=f"s2_{mt}") for mt in range(MT)]
            for mt in range(MT):
                nc.vector.memset(s1[mt][:], 0.0)
                nc.vector.memset(s2[mt][:], 0.0)

            sig9_done = None

            # ---------------- main loop ----------------
            for t in range(T):
                thr = float(9 - t)
                scale9 = t == T - 1

                # conc1 tiles for this step
                c1t = []
                for nt in range(NT):
                    hs = stage.tile([128, B * H], f32, tag="hst", bufs=6)
                    nc.sync.dma_start(
                        hs.rearrange("p (b c) -> p b c", c=H),
                        states[t, :, nt * 4096 : (nt + 1) * 4096].rearrange(
                            "b (p c) -> p b c", p=128
                        ),
                    )
                    c1 = c1p.tile([128, F1], bf16, tag="c1")
                    c1r = c1.rearrange("p (b k) -> p b k", k=H + 1)
                    if scale9:
                        nc.vector.tensor_scalar(
                            c1r[:, :, 1:],
                            hs.rearrange("p (b c) -> p b c", c=H),
                            2.0 / 3.0,
                            None,
                            op0=Alu.mult,
                        )
                        nc.vector.tensor_scalar(
                            c1r[:, :, 0:1],
                            xall[nt][:, t * B : (t + 1) * B].rearrange(
                                "p (b o) -> p b o", o=1
                            ),
                            2.0 / 3.0,
                            None,
                            op0=Alu.mult,
                        )
                    else:
                        nc.vector.tensor_copy(
                            c1r[:, :, 1:], hs.rearrange("p (b c) -> p b c", c=H)
                        )
                        nc.vector.tensor_copy(
                            c1r[:, :, 0:1],
                            xall[nt][:, t * B : (t + 1) * B].rearrange(
                                "p (b o) -> p b o", o=1
                            ),
                        )
                    c1t.append(c1)

                # masked combined M tiles + conv1 matmuls
                z1a = z1p.tile([128, 2 * F1], f32, tag="z1a", name="z1a")
                z1b = z1p.tile([128, 2 * F1], f32, tag="z1b", name="z1b")
                z1h = [z1a, z1b]
                mtiles = []
                for nt in range(NT):
                    mk = mtmp.tile([128, RPC], bf16, tag="mk")
                    nc.vector.scalar_tensor_tensor(
                        mk[:], td_bf[nt][:], thr, dtw_bf[nt][:],
                        op0=Alu.is_gt, op1=Alu.mult,
                    )
                    mtl = mpool.tile([128, RPC], bf16, tag="m")
                    nc.vector.tensor_add(mtl[:], mk[:], sle_bf[nt][:])
                    if scale9:
                        lstg = stage.tile([128, RPC], f32, tag="stg")
                        nc.sync.dma_start(lstg[:], lapT[nt * 128 : (nt + 1) * 128, :])
                        lbf = mtmp.tile([128, RPC], bf16, tag="mk")
                        nc.vector.tensor_copy(lbf[:], lstg[:])
                        mtl2 = mpool.tile([128, RPC], bf16, tag="m")
                        nc.vector.tensor_add(mtl2[:], mtl[:], lbf[:])
                        mtl = mtl2
                    mtiles.append(mtl)
                    if dbg and t == 0 and nt == 0:
                        nc.sync.dma_start(dbg_td[:], td_bf[0][:])
                        nc.sync.dma_start(dbg_m[:], mtl[:])
                    for mt in range(MT):
                        # start only once per PSUM bank: the bank-wide
                        # has_written reset would wipe the sibling slice's
                        # accumulation state otherwise.
                        nc.tensor.matmul(
                            z1h[mt // 2][:, (mt % 2) * F1 : (mt % 2 + 1) * F1],
                            mtl[:, mt * 128 : (mt + 1) * 128],
                            c1t[nt][:],
                            start=(nt == 0 and mt % 2 == 0),
                            stop=(nt == NT - 1),
                        )

                if dbg and t == 0:
                    zdbg = wk.tile([128, 2 * F1], f32, tag="zdbg", bufs=2, name="zdbg")
                    nc.vector.tensor_copy(zdbg[:], z1a[:])
                    nc.sync.dma_start(dbg_z1[:], zdbg[:])
                # per-m-tile: update S1, small matmul to gcn1, sigmoid, send to AG
                biasf = wk.tile([128, 256], f32, tag="biasf", bufs=2, name="biasf")
                nc.sync.dma_start(biasf[:], biastab[t : t + 1, :].broadcast_to((128, 256)))
                for mt in range(MT):
                    s1n = accp.tile([128, F1], f32, tag=f"s1_{mt}")
                    nc.vector.tensor_add(
                        s1n[:], s1[mt][:],
                        z1h[mt // 2][:, (mt % 2) * F1 : (mt % 2 + 1) * F1],
                    )
                    s1[mt] = s1n
                    tb = wk.tile([128, F1], f32, tag="tbf")
                    nc.vector.tensor_copy(tb[:], s1n[:])
                    g1 = g1p.tile([128, B * 2 * H], f32, tag="g1")
                    for b in range(B):
                        tz = tpzp.tile([H + 1, 128], f32, tag="tz")
                        nc.tensor.transpose(
                            tz[:], tb[:, b * (H + 1) : (b + 1) * (H + 1)], ident[:]
                        )
                        zbt = wk.tile([H + 1, 128], f32, tag="zbt")
                        nc.scalar.copy(zbt[:], tz[:])
                        nc.tensor.matmul(
                            g1[:, b * 2 * H : (b + 1) * 2 * H],
                            zbt[:],
                            w1s[:],
                            start=True,
                            stop=True,
                        )
                    sigi = wk.tile([128, B * 2 * H], f32, tag="sigi")
                    nc.vector.scalar_tensor_tensor(
                        sigi[:], g1[:], 1.0,
                        biasf[:],
                        op0=Alu.mult, op1=Alu.add,
                    )
                    sigb = wk.tile([128, B * 2 * H], bf16, tag="sigb")
                    nc.scalar.activation(sigb[:], sigi[:], Act.Sigmoid)
                    if dbg and t == 0 and mt == 0:
                        nc.sync.dma_start(dbg_sig[:], sigb[:])
                        gdbg = wk.tile([128, 256], f32, tag="gdbg", bufs=1, name="gdbg")
                        nc.vector.tensor_copy(gdbg[:], g1[:])
                        nc.sync.dma_start(dbg_g1[:], gdbg[:])
                    nc.sync.dma_start(
                        agsrc[t][:, mt * 128 * 2 * H : (mt + 1) * 128 * 2 * H]
                        .rearrange("b (p o) -> p b o", p=128),
                        sigb.rearrange("p (b o) -> p b o", o=2 * H),
                    )

                # all-gather the sigmoid values
                nc.gpsimd.collective_compute(
                    "AllGather",
                    Alu.bypass,
                    replica_groups=[CORES],
                    ins=[agsrc[t][:]],
                    outs=[agdst[t][:]],
                )

                if dbg and t == 0:
                    nc.sync.dma_start(dbg_ag[:], agdst[0][:])
                # conv2: gated conc, matmuls
                z2a = z2p.tile([128, 2 * F1], f32, tag="z2a", name="z2a")
                z2b = z2p.tile([128, 2 * F1], f32, tag="z2b", name="z2b")
                z2h = [z2a, z2b]
                for nt in range(NT):
                    rl = rp.tile([128, B * H], bf16, tag="rl")
                    nc.sync.dma_start(
                        rl.rearrange("p (b c) -> p b c", c=H),
                        agdst[t][nt // 8, :, (nt % 8) * 4096 : (nt % 8) * 4096 + 4096]
                        .rearrange("b (p c) -> p b c", p=128),
                    )
                    hs2 = stage.tile([128, B * H], f32, tag="hst", bufs=6, name="hs2")
                    nc.sync.dma_start(
                        hs2.rearrange("p (b c) -> p b c", c=H),
                        states[t, :, nt * 4096 : (nt + 1) * 4096].rearrange(
                            "b (p c) -> p b c", p=128
                        ),
                    )
                    hb = c2p.tile([128, B * H], bf16, tag="hb")
                    if scale9:
                        nc.vector.tensor_scalar(hb[:], hs2[:], 2.0 / 3.0, None, op0=Alu.mult)
                    else:
                        nc.vector.tensor_copy(hb[:], hs2[:])
                    c2 = c2p.tile([128, F1], bf16, tag="c2")
                    c2r = c2.rearrange("p (b k) -> p b k", k=H + 1)
                    nc.vector.tensor_mul(
                        c2r[:, :, 1:],
                        rl.rearrange("p (b c) -> p b c", c=H),
                        hb.rearrange("p (b c) -> p b c", c=H),
                    )
                    if scale9:
                        nc.vector.tensor_scalar(
                            c2r[:, :, 0:1],
                            xall[nt][:, t * B : (t + 1) * B].rearrange(
                                "p (b o) -> p b o", o=1
                            ),
                            2.0 / 3.0, None, op0=Alu.mult,
                        )
                    else:
                        nc.vector.tensor_copy(
                            c2r[:, :, 0:1],
                            xall[nt][:, t * B : (t + 1) * B].rearrange(
                                "p (b o) -> p b o", o=1
                            ),
                        )
                    if dbg and t == 0 and nt == 0:
                        nc.sync.dma_start(dbg_c2[:], c2[:])
                    for mt in range(MT):
                        nc.tensor.matmul(
                            z2h[mt // 2][:, (mt % 2) * F1 : (mt % 2 + 1) * F1],
                            mtiles[nt][:, mt * 128 : (mt + 1) * 128],
                            c2[:],
                            start=(nt == 0 and mt % 2 == 0),
                            stop=(nt == NT - 1),
                        )

                if dbg and t == 0:
                    zdbg2 = wk.tile([128, 2 * F1], f32, tag="zdbg", bufs=2, name="zdbg2")
                    nc.vector.tensor_copy(zdbg2[:], z2a[:])
                    nc.sync.dma_start(dbg_z2[:], zdbg2[:])
                for mt in range(MT):
                    s2n = accp.tile([128, F1], f32, tag=f"s2_{mt}")
                    nc.vector.tensor_add(
                        s2n[:], s2[mt][:],
                        z2h[mt // 2][:, (mt % 2) * F1 : (mt % 2 + 1) * F1],
                    )
                    s2[mt] = s2n

            # ---------------- tail: tanh(gcn2), aux AG, output ----------------
            bias2f = wk.tile([128, B * H], f32, tag="bias2f", bufs=1, name="bias2f")
            nc.sync.dma_start(bias2f[:], biastab[10 : 11, : B * H].broadcast_to((128, B * H)))
            for mt in range(MT):
                tb2 = wk.tile([128, F1], f32, tag="tbf")
                nc.vector.tensor_copy(tb2[:], s2[mt][:])
                g2 = g1p.tile([128, B * H], f32, tag="g1")
                for b in range(B):
                    tz = tpzp.tile([H + 1, 128], f32, tag="tz")
                    nc.tensor.transpose(
                        tz[:], tb2[:, b * (H + 1) : (b + 1) * (H + 1)], ident[:]
                    )
                    zbt = wk.tile([H + 1, 128], f32, tag="zbt")
                    nc.scalar.copy(zbt[:], tz[:])
                    nc.tensor.matmul(
                        g2[:, b * H : (b + 1) * H], zbt[:], w2s[:],
                        start=True, stop=True,
                    )
                tani = wk.tile([128, B * H], f32, tag="sigi")
                nc.vector.scalar_tensor_tensor(
                    tani[:], g2[:], 1.0,
                    bias2f[:],
                    op0=Alu.mult, op1=Alu.add,
                )
                tanb = wk.tile([128, B * H], bf16, tag="sigb")
                nc.scalar.activation(tanb[:], tani[:], Act.Tanh)
                nc.sync.dma_start(
                    auxsrc[:, mt * 128 * H : (mt + 1) * 128 * H]
                    .rearrange("b (p c) -> p b c", p=128),
                    tanb.rearrange("p (b c) -> p b c", c=H),
                )
            nc.gpsimd.collective_compute(
                "AllGather",
                Alu.bypass,
                replica_groups=[CORES],
                ins=[auxsrc[:]],
                outs=[auxdst[:]],
            )

            # output: every core computes the full [B, N*H]
            for nt in range(NT):
                ul = rp.tile([128, B * H], bf16, tag="ul")
                nc.sync.dma_start(
                    ul.rearrange("p (b c) -> p b c", c=H),
                    agdst[T - 1][4 + nt // 8, :,
                                 (nt % 8) * 4096 : (nt % 8) * 4096 + 4096]
                    .rearrange("b (p c) -> p b c", p=128),
                )
                cl = rp.tile([128, B * H], bf16, tag="cl")
                nc.sync.dma_start(
                    cl.rearrange("p (b c) -> p b c", c=H),
                    auxdst[nt // 4, :, (nt % 4) * 4096 : (nt % 4) * 4096 + 4096]
                    .rearrange("b (p c) -> p b c", p=128),
                )
                h9 = stage.tile([128, B * H], f32, tag="hst", bufs=6)
                nc.sync.dma_start(
                    h9.rearrange("p (b c) -> p b c", c=H),
                    states[T - 1, :, nt * 4096 : (nt + 1) * 4096].rearrange(
                        "b (p c) -> p b c", p=128
                    ),
                )
                cf = wk.tile([128, B * H], f32, tag="cf", bufs=2)
                nc.vector.tensor_copy(cf[:], cl[:])
                uf = wk.tile([128, B * H], f32, tag="uf", bufs=2)
                nc.vector.tensor_copy(uf[:], ul[:])
                dd = wk.tile([128, B * H], f32, tag="dd", bufs=2)
                nc.vector.tensor_sub(dd[:], h9[:], cf[:])
                mm = wk.tile([128, B * H], f32, tag="mmv", bufs=2)
                nc.vector.tensor_mul(mm[:], uf[:], dd[:])
                ho = wk.tile([128, B * H], f32, tag="ho", bufs=2)
                nc.vector.tensor_add(ho[:], mm[:], cf[:])
                nc.sync.dma_start(
                    hout[:, nt * 4096 : (nt + 1) * 4096].rearrange(
                        "b (p c) -> p b c", p=128
                    ),
                    ho.rearrange("p (b c) -> p b c", c=H),
                )

    nc.finalize()
    return nc


_NC_CACHE = None


def _get_nc(dbg=False):
    global _NC_CACHE
    if _NC_CACHE is None:
        _NC_CACHE = _build_nc(dbg)
    return _NC_CACHE


def make_in_maps(inputs, states, dtw, spec_lap, laplacian, time_delay,
                 W1, b1, W2, b2):
    dtwT = np.ascontiguousarray(dtw.T)
    tdT = np.ascontiguousarray(time_delay.T)
    sleT = np.ascontiguousarray(spec_lap.T)
    lapT = np.ascontiguousarray(laplacian.T)
    xT = np.ascontiguousarray(inputs.transpose(1, 2, 0))        # [T, N, B]
    states = np.ascontiguousarray(states)
    w1hv = (0.5 * W1).astype(np.float32)
    w2hv = (0.5 * W2).astype(np.float32)
    bt = np.zeros((11, 256), np.float32)
    for t in range(T):
        bt[t] = np.tile((t + 1.0) * b1, B)
    bt[10, : B * H] = np.tile(10.0 * b2, B)

    in_maps = []
    for c in range(NC):
        blk = slice(c * RPC, (c + 1) * RPC)
        sle_c = np.ascontiguousarray(sleT[:, blk])
        # add identity: global row n == column (local) m  ->  n = c*RPC + m
        idx = np.arange(RPC)
        sle_c[c * RPC + idx, idx] += 1.0
        in_maps.append(
            dict(
                dtwT=np.ascontiguousarray(dtwT[:, blk]),
                tdT=np.ascontiguousarray(tdT[:, blk]),
                sleT=sle_c,
                lapT=np.ascontiguousarray(lapT[:, blk]),
                states=states,
                xT=xT,
                w1h=w1hv,
                w2h=w2hv,
                biastab=bt,
            )
        )
    return in_maps


def kernel(inputs, states, dtw, spec_lap, laplacian, time_delay,
           W1, b1, W2, b2):
    in_maps = make_in_maps(
        np.asarray(inputs, np.float32), np.asarray(states, np.float32),
        np.asarray(dtw, np.float32), np.asarray(spec_lap, np.float32),
        np.asarray(laplacian, np.float32), np.asarray(time_delay, np.float32),
        np.asarray(W1, np.float32), np.asarray(b1, np.float32),
        np.asarray(W2, np.float32), np.asarray(b2, np.float32),
    )
    nc = _get_nc()
    res = run_bass_kernel_spmd(nc, in_maps, CORES, trace=False)
    return np.asarray(res.results[0]["hout"], np.float32)



# revision 3
# speedup vs baseline: 1.7338x; 1.7338x over previous
"""FESTGCN Trainium2 kernel: 8-core SPMD Bass/Tile implementation (v2).

Algorithm (validated against the reference in numpy, see sim_check.py):
  For t in 0..9:
    M_t = dtw * (ceil|td| > 9-t) + (spec_lap + I)   [t=9: + laplacian, x2/3]
    S1 += M_t @ c1_t,   c1_t = [x_t | h_t]
    gcn1_t = 0.5 * S1 @ W1 + (t+1) b1 ;  sig_t = sigmoid(gcn1_t)
    r_t = first flat half of sig_t ;  c2_t = [x_t | r_t*h_t]
    S2 += M_t @ c2_t
  u = second flat half of sig_9 ; c = tanh(0.5 * S2 @ W2 + 10 b2)
  out = u*h_9 + (1-u)*c        (final mix done on host)

Key structure vs v1:
  * Contraction (node) axis globally permuted to sigma order: first all even
    c2-nodes (as "gnodes" 0..2047), then all odd ones.  With the reference's
    flat-split gating, r for c2-node 2g+p is sig[gnode g, ch 32p:32p+32], so
    every gather read becomes a contiguous row-slice - no transposing DMAs.
  * Each core owns rows pi_c = [256c,256c+256) u [2048+256c,2048+256c+256):
    the first sub-block produces the r-side sigmoids (AllGathered per step,
    overlapped with compute), the second the u-side (device output at t=9).
  * Two phases: all conv1 steps (with the sigmoid chain software-pipelined one
    step behind the matmuls), then all conv2 steps.  AllGather latency hides
    under compute.
  * t=9 matrix (masked dtw + spec_lap + I + laplacian) is fully baked on the
    host - no laplacian load, no t=9 mask generation.
  * c1 tiles are host-built; h/x are host-laid-out so every DMA is contiguous.
  * Final gating mix u*h9+(1-u)*c runs on the host (cheap), removing the aux
    AllGather and the redundant full-output tail.
"""

import numpy as np

import concourse.bacc as bacc
import concourse.mybir as mybir
import concourse.tile as tile
from concourse.bass_utils import run_bass_kernel_spmd

B, T, N, H = 4, 10, 4096, 32
NC = 8
RPC = N // NC            # 512 rows per core
NT = N // 128            # 32 contraction tiles
MT = RPC // 128          # 4 m-tiles per core
F1 = B * (H + 1)         # 132 moving columns per conv
NH = N * H
f32 = mybir.dt.float32
f16 = mybir.dt.float16
Alu = mybir.AluOpType
Act = mybir.ActivationFunctionType
CORES = list(range(NC))

PERM = np.concatenate([np.arange(0, N, 2), np.arange(1, N, 2)])
IPERM = np.empty(N, np.int64)
IPERM[PERM] = np.arange(N)


def _rows_of(c):
    return np.concatenate(
        [np.arange(256 * c, 256 * c + 256),
         np.arange(2048 + 256 * c, 2048 + 256 * c + 256)]
    )


def _build_nc():
    nc = bacc.Bacc(
        "TRN2",
        target_bir_lowering=False,
        debug=False,
        enable_asserts=False,
        num_devices=NC,
    )
    dtwT = nc.dram_tensor("dtwT", [N, RPC], f16, kind="ExternalInput").ap()
    ctT = nc.dram_tensor("ctT", [N, RPC], f16, kind="ExternalInput").ap()
    sleT = nc.dram_tensor("sleT", [N, RPC], f16, kind="ExternalInput").ap()
    a9T = nc.dram_tensor("a9T", [N, RPC], f16, kind="ExternalInput").ap()
    c1all = nc.dram_tensor("c1all", [T, N, F1], f16, kind="ExternalInput").ap()
    hall = nc.dram_tensor("hall", [T, N, B * H], f16, kind="ExternalInput").ap()
    xall = nc.dram_tensor("xall", [N, T * B], f16, kind="ExternalInput").ap()
    w1h = nc.dram_tensor("w1h", [H + 1, 2 * H], f32, kind="ExternalInput").ap()
    w2h = nc.dram_tensor("w2h", [H + 1, H], f32, kind="ExternalInput").ap()
    biastab = nc.dram_tensor("biastab", [11, 256], f32, kind="ExternalInput").ap()
    hout = nc.dram_tensor("hout", [RPC, B * H], f32, kind="ExternalOutput").ap()
    sig9 = nc.dram_tensor("sig9", [256, B * 2 * H], f16, kind="ExternalOutput").ap()

    with tile.TileContext(nc) as tc:
        with (
            tc.tile_pool(name="big", bufs=1) as big,        # resident f16 matrices
            tc.tile_pool(name="sm", bufs=1) as sm,          # small persistents
            tc.tile_pool(name="mtmp", bufs=4) as mtmp,      # mask intermediates
            tc.tile_pool(name="mpool", bufs=6) as mpool,    # combined M tiles
            tc.tile_pool(name="c1p", bufs=6) as c1p,
            tc.tile_pool(name="hp", bufs=6) as hp,
            tc.tile_pool(name="c2p", bufs=6) as c2p,
            tc.tile_pool(name="rlp", bufs=32) as rlp,       # gathered sigmoids
            tc.tile_pool(name="acc", bufs=3) as accp,       # S1/S2 rings
            tc.tile_pool(name="wk", bufs=3) as wk,
            tc.tile_pool(name="zp", bufs=1, space="PSUM") as zp,
            tc.tile_pool(name="tpz", bufs=2, space="PSUM") as tpzp,
            tc.tile_pool(name="g1p", bufs=2, space="PSUM") as g1p,
            tc.tile_pool(name="dramp", bufs=1, space="DRAM") as dramp,
        ):
            agsrc = [
                dramp.tile([256, B * 2 * H], f16, tag=f"agsrc{t}",
                           name=f"agsrc{t}")
                for t in range(T)
            ]
            agdst = [
                dramp.tile([NC * 256, B * 2 * H], f16, tag=f"agdst{t}",
                           name=f"agdst{t}", addr_space="Shared")
                for t in range(T)
            ]

            # ---------------- prologue ----------------
            iota_i = wk.tile([128, 128], mybir.dt.int32, tag="iota", bufs=1)
            nc.gpsimd.iota(iota_i[:], pattern=[[1, 128]], base=0,
                           channel_multiplier=-1)
            ident = sm.tile([128, 128], f32, tag="ident")
            nc.vector.tensor_scalar(ident[:], iota_i[:], 0, None,
                                    op0=Alu.is_equal)
            w1s = sm.tile([H + 1, 2 * H], f32, tag="w1s")
            nc.sync.dma_start(w1s[:], w1h[:])
            w2s = sm.tile([H + 1, H], f32, tag="w2s")
            nc.sync.dma_start(w2s[:], w2h[:])

            dtw_bf, ct_bf, sle_bf = [], [], []
            for nt in range(NT):
                sl = slice(nt * 128, nt * 128 + 128)
                tb = big.tile([128, RPC], f16, tag=f"ct{nt}")
                nc.sync.dma_start(tb[:], ctT[sl, :])
                ct_bf.append(tb)
                tb = big.tile([128, RPC], f16, tag=f"dtw{nt}")
                nc.sync.dma_start(tb[:], dtwT[sl, :])
                dtw_bf.append(tb)
                tb = big.tile([128, RPC], f16, tag=f"sle{nt}")
                nc.sync.dma_start(tb[:], sleT[sl, :])
                sle_bf.append(tb)
            a9_bf = []
            for nt in range(NT):
                sl = slice(nt * 128, nt * 128 + 128)
                tb = big.tile([128, RPC], f16, tag=f"a9{nt}")
                nc.sync.dma_start(tb[:], a9T[sl, :])
                a9_bf.append(tb)
            xall_s = []
            for nt in range(NT):
                sl = slice(nt * 128, nt * 128 + 128)
                xb = sm.tile([128, T * B], f16, tag=f"x{nt}")
                nc.sync.dma_start(xb[:], xall[sl, :])
                xall_s.append(xb)

            s1 = [accp.tile([128, F1], f32, tag=f"s1_{mt}", name=f"s1_{mt}")
                  for mt in range(MT)]
            s2 = [accp.tile([128, F1], f32, tag=f"s2_{mt}", name=f"s2_{mt}")
                  for mt in range(MT)]
            for mt in range(MT):
                nc.vector.memset(s1[mt][:], 0.0)
                nc.vector.memset(s2[mt][:], 0.0)

            def mask_tile(t, nt):
                if t == 9:
                    return a9_bf[nt]
                mk = mtmp.tile([128, RPC], f16, tag="mk")
                nc.vector.scalar_tensor_tensor(
                    mk[:], ct_bf[nt][:], float(9 - t), dtw_bf[nt][:],
                    op0=Alu.is_gt, op1=Alu.mult,
                )
                m = mpool.tile([128, RPC], f16, tag="m")
                nc.vector.tensor_add(m[:], mk[:], sle_bf[nt][:])
                return m

            def zpair(t):
                tags = ("z1a", "z1b") if t % 2 == 0 else ("z2a", "z2b")
                return [zp.tile([128, 2 * F1], f32, tag=tg, name=f"{tg}_{t}")
                        for tg in tags]

            def chain1(t, s1t):
                """S1(t) -> gcn1 -> sigmoid -> agsrc[t] (+ sig9 at t=9)."""
                mts = (0, 1, 2, 3) if t == 9 else (0, 1)
                biasf = wk.tile([128, 256], f32, tag="biasf", bufs=3)
                nc.sync.dma_start(
                    biasf[:], biastab[t : t + 1, :].broadcast_to((128, 256))
                )
                for mt in mts:
                    tb = wk.tile([128, F1], f32, tag="tbf", bufs=4)
                    nc.vector.tensor_copy(tb[:], s1t[mt][:])
                    g1 = g1p.tile([128, 256], f32, tag="g1")
                    for b in range(B):
                        tz = tpzp.tile([H + 1, 128], f32, tag="tz")
                        nc.tensor.transpose(
                            tz[:], tb[:, b * (H + 1) : (b + 1) * (H + 1)],
                            ident[:],
                        )
                        zbt = wk.tile([H + 1, 128], f32, tag="zbt", bufs=4)
                        nc.scalar.copy(zbt[:], tz[:])
                        nc.tensor.matmul(
                            g1[:, b * 2 * H : (b + 1) * 2 * H], zbt[:], w1s[:],
                            start=True, stop=True,
                        )
                    sigi = wk.tile([128, 256], f32, tag="sigi", bufs=4)
                    nc.vector.scalar_tensor_tensor(
                        sigi[:], g1[:], 1.0, biasf[:], op0=Alu.mult, op1=Alu.add
                    )
                    sigb = wk.tile([128, 256], f16, tag="sigb", bufs=4)
                    nc.scalar.activation(sigb[:], sigi[:], Act.Sigmoid)
                    if mt < 2:
                        nc.sync.dma_start(
                            agsrc[t][mt * 128 : (mt + 1) * 128, :], sigb[:]
                        )
                    else:
                        nc.sync.dma_start(
                            sig9[(mt - 2) * 128 : (mt - 1) * 128, :], sigb[:]
                        )
                nc.gpsimd.collective_compute(
                    "AllGather",
                    Alu.bypass,
                    replica_groups=[CORES],
                    ins=[agsrc[t][:]],
                    outs=[agdst[t][:]],
                )

            # ---------------- phase 1 ----------------
            s1hist = []
            for t in range(T):
                zh = zpair(t)
                for nt in range(NT):
                    c1 = c1p.tile([128, F1], f16, tag="c1")
                    nc.scalar.dma_start(
                        c1[:], c1all[t, nt * 128 : (nt + 1) * 128, :]
                    )
                    m = mask_tile(t, nt)
                    for mt in range(MT):
                        nc.tensor.matmul(
                            zh[mt // 2][:, (mt % 2) * F1 : (mt % 2 + 1) * F1],
                            m[:, mt * 128 : (mt + 1) * 128],
                            c1[:],
                            start=(nt == 0 and mt % 2 == 0),
                            stop=(nt == NT - 1),
                        )
                s1t = []
                for mt in range(MT):
                    s1n = accp.tile([128, F1], f32, tag=f"s1_{mt}")
                    nc.vector.tensor_add(
                        s1n[:], s1[mt][:],
                        zh[mt // 2][:, (mt % 2) * F1 : (mt % 2 + 1) * F1],
                    )
                    s1[mt] = s1n
                    s1t.append(s1n)
                s1hist.append(s1t)
                if t >= 1:
                    chain1(t - 1, s1hist[t - 1])
            chain1(T - 1, s1hist[T - 1])

            # ---------------- phase 2 ----------------
            for t in range(T):
                rl = []
                for gt in range(16):
                    r = rlp.tile([128, B * 2 * H], f16, tag="rl")
                    nc.scalar.dma_start(
                        r[:], agdst[t][gt * 128 : (gt + 1) * 128, :]
                    )
                    rl.append(r)
                zh = zpair(t)
                for nt in range(NT):
                    hs = hp.tile([128, B * H], f16, tag="hs")
                    nc.scalar.dma_start(
                        hs[:], hall[t, nt * 128 : (nt + 1) * 128, :]
                    )
                    gt, ch0 = (nt, 0) if nt < 16 else (nt - 16, H)
                    c2 = c2p.tile([128, F1], f16, tag="c2")
                    c2r = c2.rearrange("p (b k) -> p b k", k=H + 1)
                    nc.vector.tensor_mul(
                        c2r[:, :, 1:],
                        rl[gt].rearrange("p (b c) -> p b c", c=2 * H)[
                            :, :, ch0 : ch0 + H
                        ],
                        hs.rearrange("p (b c) -> p b c", c=H),
                    )
                    nc.vector.tensor_copy(
                        c2r[:, :, 0:1],
                        xall_s[nt][:, t * B : (t + 1) * B].rearrange(
                            "p (b o) -> p b o", o=1
                        ),
                    )
                    m = mask_tile(t, nt)
                    for mt in range(MT):
                        nc.tensor.matmul(
                            zh[mt // 2][:, (mt % 2) * F1 : (mt % 2 + 1) * F1],
                            m[:, mt * 128 : (mt + 1) * 128],
                            c2[:],
                            start=(nt == 0 and mt % 2 == 0),
                            stop=(nt == NT - 1),
                        )
                for mt in range(MT):
                    s2n = accp.tile([128, F1], f32, tag=f"s2_{mt}")
                    nc.vector.tensor_add(
                        s2n[:], s2[mt][:],
                        zh[mt // 2][:, (mt % 2) * F1 : (mt % 2 + 1) * F1],
                    )
                    s2[mt] = s2n

            # ---------------- tail ----------------
            bias2f = wk.tile([128, B * H], f32, tag="bias2f", bufs=1)
            nc.sync.dma_start(
                bias2f[:], biastab[10 : 11, : B * H].broadcast_to((128, B * H))
            )
            for mt in range(MT):
                tb2 = wk.tile([128, F1], f32, tag="tbf", bufs=4)
                nc.vector.tensor_copy(tb2[:], s2[mt][:])
                g2 = g1p.tile([128, 256], f32, tag="g1")
                for b in range(B):
                    tz = tpzp.tile([H + 1, 128], f32, tag="tz")
                    nc.tensor.transpose(
                        tz[:], tb2[:, b * (H + 1) : (b + 1) * (H + 1)], ident[:]
                    )
                    zbt = wk.tile([H + 1, 128], f32, tag="zbt", bufs=4)
                    nc.scalar.copy(zbt[:], tz[:])
                    nc.tensor.matmul(
                        g2[:, b * H : (b + 1) * H], zbt[:], w2s[:],
                        start=True, stop=True,
                    )
                tani = wk.tile([128, B * H], f32, tag="tani", bufs=2)
                nc.vector.scalar_tensor_tensor(
                    tani[:], g2[:, : B * H], 1.0, bias2f[:],
                    op0=Alu.mult, op1=Alu.add,
                )
                tanf = wk.tile([128, B * H], f32, tag="tanf", bufs=2)
                nc.scalar.activation(tanf[:], tani[:], Act.Tanh)
                nc.sync.dma_start(hout[mt * 128 : (mt + 1) * 128, :], tanf[:])

    nc.finalize()
    return nc


_NC_CACHE = None


def _get_nc():
    global _NC_CACHE
    if _NC_CACHE is None:
        _NC_CACHE = _build_nc()
    return _NC_CACHE


def make_in_maps(inputs, states, dtw, spec_lap, laplacian, time_delay,
                 W1, b1, W2, b2):
    inputs = np.asarray(inputs, np.float32)
    states = np.asarray(states, np.float32)
    dtw = np.asarray(dtw, np.float32)
    spec_lap = np.asarray(spec_lap, np.float32)
    laplacian = np.asarray(laplacian, np.float32)
    time_delay = np.asarray(time_delay, np.float32)
    W1 = np.asarray(W1, np.float32)
    b1 = np.asarray(b1, np.float32)
    W2 = np.asarray(W2, np.float32)
    b2 = np.asarray(b2, np.float32)

    ct_full = np.ceil(np.abs(time_delay))
    # G matrices: [sigma-node, row] layout, fp16
    Gdtw = np.ascontiguousarray(dtw[:, PERM].T, np.float16)
    Gct = np.ascontiguousarray(ct_full[:, PERM].T, np.float16)
    Gsle = dtw  # placeholder to free name; real below
    sle_p = spec_lap[:, PERM].T.copy()          # [sigma, row] f32
    sle_p[IPERM, np.arange(N)] += 1.0           # + identity in sigma coords
    Gsle = sle_p.astype(np.float16)
    mdtw9 = np.where(ct_full >= 1.0, dtw, 0.0)
    a9_p = (mdtw9 + spec_lap + laplacian)[:, PERM].T.copy()
    a9_p[IPERM, np.arange(N)] += 1.0
    Ga9 = a9_p.astype(np.float16)

    x = inputs.transpose(1, 0, 2)               # [T, B, N]
    h = states.reshape(T, B, N, H)
    conc = np.concatenate([x[:, :, :, None], h], axis=3)  # [T,B,N,33]
    conc[9] *= 2.0 / 3.0
    concp = conc.transpose(0, 2, 1, 3)[:, PERM]  # [T, N(sigma), B, 33]
    c1all = np.ascontiguousarray(
        concp.reshape(T, N, F1), np.float16
    )
    hall = np.ascontiguousarray(
        concp[:, :, :, 1:].reshape(T, N, B * H), np.float16
    )
    xall = np.ascontiguousarray(
        concp[:, :, :, 0].transpose(1, 0, 2).reshape(N, T * B), np.float16
    )

    w1hv = (0.5 * W1).astype(np.float32)
    w2hv = (0.5 * W2).astype(np.float32)
    bt = np.zeros((11, 256), np.float32)
    for t in range(T):
        bt[t] = np.tile((t + 1.0) * b1, B)
    bt[10, : B * H] = np.tile(10.0 * b2, B)

    in_maps = []
    for c in range(NC):
        rows = _rows_of(c)
        in_maps.append(
            dict(
                dtwT=np.ascontiguousarray(Gdtw[:, rows]),
                ctT=np.ascontiguousarray(Gct[:, rows]),
                sleT=np.ascontiguousarray(Gsle[:, rows]),
                a9T=np.ascontiguousarray(Ga9[:, rows]),
                c1all=c1all,
                hall=hall,
                xall=xall,
                w1h=w1hv,
                w2h=w2hv,
                biastab=bt,
            )
        )
    return in_maps


def kernel(inputs, states, dtw, spec_lap, laplacian, time_delay,
           W1, b1, W2, b2):
    states = np.asarray(states, np.float32)
    in_maps = make_in_maps(
        inputs, states, dtw, spec_lap, laplacian, time_delay, W1, b1, W2, b2
    )
    nc = _get_nc()
    res = run_bass_kernel_spmd(nc, in_maps, CORES, trace=False)

    cmat = np.empty((N, B, H), np.float32)
    umat = np.empty((2048, B, 2 * H), np.float32)
    for c in range(NC):
        rows = _rows_of(c)
        cmat[rows] = np.asarray(
            res.results[c]["hout"], np.float32
        ).reshape(RPC, B, H)
        umat[256 * c : 256 * c + 256] = np.asarray(
            res.results[c]["sig9"], np.float32
        ).reshape(256, B, 2 * H)
    u = umat.transpose(1, 0, 2).reshape(B, NH)
    cfl = cmat.transpose(1, 0, 2).reshape(B, NH)
    h9 = states[T - 1]
    return (u * h9 + (1.0 - u) * cfl).astype(np.float32)


# revision 10
# speedup vs baseline: 2.4483x; 1.4121x over previous
"""FESTGCN Trainium2 kernel: 8-core SPMD Bass/Tile implementation (v3).

Algorithm (validated against the reference in numpy, see sim_check.py):
  For t in 0..9:
    M_t = dtw * (ceil|td| > 9-t) + (spec_lap + I)   [t=9: + laplacian, x2/3]
    S1 += M_t @ c1_t,   c1_t = [x_t | h_t]
    gcn1_t = 0.5 * S1 @ W1 + (t+1) b1 ;  sig_t = sigmoid(gcn1_t)
    r_t = first flat half of sig_t ;  c2_t = [x_t | r_t*h_t]
    S2 += M_t @ c2_t
  u = second flat half of sig_9 ; c = tanh(0.5 * S2 @ W2 + 10 b2)
  out = u*h_9 + (1-u)*c        (final mix done on host)

Structure:
  * All ten masked matrices M_t are baked on the HOST (fp16, already
    transposed/sliced per core) - the TRN2 vector engine is far too slow
    (tensor_tensor ~214Ge/s + drains) to regenerate them per step.
  * Contraction (node) axis globally permuted to sigma order (even c2-nodes
    first), which makes every sigmoid gather a contiguous row-slice.
  * Each core owns rows pi_c = [256c,+256) u [2048+256c,+256): first
    sub-block feeds the per-step AllGather of r-side sigmoids, second is the
    u-side (device output at t=9 only).
  * Single interleaved pass: conv2(t) runs two outer steps behind conv1(t),
    so each M_t tile is DMA'd once and held ~3 steps in SBUF; the AllGather
    latency hides under the intervening compute.
  * c2's x-column is applied by a tiny 4-column matmul sharing the stationary
    M-tile (no per-tile DVE copy); c2's h operand is read out of the c1 tile.
  * Final gating mix on host (no aux collective, no redundant full output).
"""

import numpy as np

import concourse.bacc as bacc
import concourse.mybir as mybir
import concourse.tile as tile
from concourse.bass_utils import run_bass_kernel_spmd

B, T, N, H = 4, 10, 4096, 32
NC = 8
RPC = N // NC            # 512 rows per core
NT = N // 128            # 32 contraction tiles
MT = RPC // 128          # 4 m-tiles per core
F1 = B * (H + 1)         # 132 moving columns per conv
F2 = B * H               # 128 rh columns
NH = N * H
DELAY = 2                # conv2 runs this many outer steps behind conv1
f32 = mybir.dt.float32
f16 = mybir.dt.float16
Alu = mybir.AluOpType
Act = mybir.ActivationFunctionType
CORES = list(range(NC))

PERM = np.concatenate([np.arange(0, N, 2), np.arange(1, N, 2)])
IPERM = np.empty(N, np.int64)
IPERM[PERM] = np.arange(N)


def _rows_of(c):
    return np.concatenate(
        [np.arange(256 * c, 256 * c + 256),
         np.arange(2048 + 256 * c, 2048 + 256 * c + 256)]
    )


def _build_nc():
    nc = bacc.Bacc(
        "TRN2",
        target_bir_lowering=False,
        debug=False,
        enable_asserts=False,
        num_devices=NC,
    )
    mstack = nc.dram_tensor("mstack", [T, N, RPC], f16, kind="ExternalInput").ap()
    c1all = nc.dram_tensor("c1all", [T, N, F1], f16, kind="ExternalInput").ap()
    w1h = nc.dram_tensor("w1h", [H + 1, 2 * H], f32, kind="ExternalInput").ap()
    w2h = nc.dram_tensor("w2h", [H + 1, H], f32, kind="ExternalInput").ap()
    biastab = nc.dram_tensor("biastab", [11, 256], f32, kind="ExternalInput").ap()
    hout = nc.dram_tensor("hout", [RPC, B * H], f32, kind="ExternalOutput").ap()
    sig9 = nc.dram_tensor("sig9", [256, B * 2 * H], f16, kind="ExternalOutput").ap()

    with tile.TileContext(nc) as tc:
        with (
            tc.tile_pool(name="msp", bufs=100) as msp,      # M tiles, ~3-step hold
            tc.tile_pool(name="c1p", bufs=100) as c1p,      # c1 tiles, ~3-step hold
            tc.tile_pool(name="rlp", bufs=48) as rlp,       # gathered sigmoids
            tc.tile_pool(name="c2p", bufs=8) as c2p,
            tc.tile_pool(name="sm", bufs=1) as sm,
            tc.tile_pool(name="acc", bufs=3) as accp,
            tc.tile_pool(name="wk", bufs=3) as wk,
            tc.tile_pool(name="zp", bufs=1, space="PSUM") as zp,
            tc.tile_pool(name="tpz", bufs=2, space="PSUM") as tpzp,
            tc.tile_pool(name="g1p", bufs=2, space="PSUM") as g1p,
            tc.tile_pool(name="dramp", bufs=1, space="DRAM") as dramp,
        ):
            agsrc = [
                dramp.tile([256, B * 2 * H], f16, tag=f"agsrc{t}",
                           name=f"agsrc{t}")
                for t in range(T)
            ]
            agdst = [
                dramp.tile([NC * 256, B * 2 * H], f16, tag=f"agdst{t}",
                           name=f"agdst{t}", addr_space="Shared")
                for t in range(T)
            ]

            # ---------------- prologue ----------------
            iota_i = wk.tile([128, 128], mybir.dt.int32, tag="iota", bufs=1)
            nc.gpsimd.iota(iota_i[:], pattern=[[1, 128]], base=0,
                           channel_multiplier=-1)
            ident = sm.tile([128, 128], f32, tag="ident")
            nc.vector.tensor_scalar(ident[:], iota_i[:], 0, None,
                                    op0=Alu.is_equal)
            w1s = sm.tile([H + 1, 2 * H], f32, tag="w1s")
            nc.sync.dma_start(w1s[:], w1h[:])
            w2s = sm.tile([H + 1, H], f32, tag="w2s")
            nc.sync.dma_start(w2s[:], w2h[:])

            s1 = [accp.tile([128, F1], f32, tag=f"s1_{mt}", name=f"s1_{mt}")
                  for mt in range(MT)]
            s2h = [accp.tile([128, F2], f32, tag=f"s2h_{mt}", name=f"s2h_{mt}")
                   for mt in range(MT)]
            s2x = [accp.tile([128, B], f32, tag=f"s2x_{mt}", name=f"s2x_{mt}")
                   for mt in range(MT)]
            for mt in range(MT):
                nc.vector.memset(s1[mt][:], 0.0)
                nc.vector.memset(s2h[mt][:], 0.0)
                nc.vector.memset(s2x[mt][:], 0.0)

            ms_hold = {}
            c1_hold = {}
            rl_hold = {}

            def chain1(t, s1t):
                """S1(t) -> gcn1 -> sigmoid -> agsrc[t]/AG (+ sig9 at t=9)."""
                mts = (0, 1, 2, 3) if t == T - 1 else (0, 1)
                biasf = wk.tile([128, 256], f32, tag="biasf", bufs=3)
                nc.sync.dma_start(
                    biasf[:], biastab[t : t + 1, :].broadcast_to((128, 256))
                )
                for mt in mts:
                    tb = wk.tile([128, F1], f32, tag="tbf", bufs=4)
                    nc.vector.tensor_copy(tb[:], s1t[mt][:])
                    g1 = g1p.tile([128, 256], f32, tag="g1")
                    for b in range(B):
                        tz = tpzp.tile([H + 1, 128], f32, tag="tz")
                        nc.tensor.transpose(
                            tz[:], tb[:, b * (H + 1) : (b + 1) * (H + 1)],
                            ident[:],
                        )
                        zbt = wk.tile([H + 1, 128], f32, tag="zbt", bufs=4)
                        nc.scalar.copy(zbt[:], tz[:])
                        nc.tensor.matmul(
                            g1[:, b * 2 * H : (b + 1) * 2 * H], zbt[:], w1s[:],
                            start=True, stop=True,
                        )
                    sigi = wk.tile([128, 256], f32, tag="sigi", bufs=4)
                    nc.vector.scalar_tensor_tensor(
                        sigi[:], g1[:], 1.0, biasf[:], op0=Alu.mult, op1=Alu.add
                    )
                    sigb = wk.tile([128, 256], f16, tag="sigb", bufs=4)
                    nc.scalar.activation(sigb[:], sigi[:], Act.Sigmoid)
                    if mt < 2:
                        nc.sync.dma_start(
                            agsrc[t][mt * 128 : (mt + 1) * 128, :], sigb[:]
                        )
                    else:
                        nc.sync.dma_start(
                            sig9[(mt - 2) * 128 : (mt - 1) * 128, :], sigb[:]
                        )
                nc.gpsimd.collective_compute(
                    "AllGather",
                    Alu.bypass,
                    replica_groups=[CORES],
                    ins=[agsrc[t][:]],
                    outs=[agdst[t][:]],
                )

            def fetch_rl(t):
                # One outer step after AG(t) fired, so the scalar queue's
                # semaphore wait on the collective is already satisfied.
                rls = []
                for gt in range(16):
                    r = rlp.tile([128, B * 2 * H], f16, tag="rl", name="rl")
                    nc.scalar.dma_start(
                        r[:], agdst[t][gt * 128 : (gt + 1) * 128, :]
                    )
                    rls.append(r)
                rl_hold[t] = rls

            # ---------------- interleaved main loop ----------------
            for s in range(T + DELAY):
                if s < T:
                    t = s
                    za = zp.tile([128, 2 * F1], f32, tag="z1a", name=f"z1a_{t}")
                    zb = zp.tile([128, 2 * F1], f32, tag="z1b", name=f"z1b_{t}")
                    zh = [za, zb]
                    mss, c1s = [], []
                    for nt in range(NT):
                        m = msp.tile([128, RPC], f16, tag="ms", name="ms")
                        eng = nc.sync if nt < 24 else nc.scalar
                        eng.dma_start(
                            m[:], mstack[t, nt * 128 : (nt + 1) * 128, :]
                        )
                        c1 = c1p.tile([128, F1], f16, tag="c1", name="c1")
                        nc.scalar.dma_start(
                            c1[:], c1all[t, nt * 128 : (nt + 1) * 128, :]
                        )
                        mss.append(m)
                        c1s.append(c1)
                        for mt in range(MT):
                            nc.tensor.matmul(
                                zh[mt // 2][:, (mt % 2) * F1 : (mt % 2 + 1) * F1],
                                m[:, mt * 128 : (mt + 1) * 128],
                                c1[:],
                                start=(nt == 0 and mt % 2 == 0),
                                stop=(nt == NT - 1),
                            )
                    ms_hold[t] = mss
                    c1_hold[t] = c1s
                    s1t = []
                    for mt in range(MT):
                        s1n = accp.tile([128, F1], f32, tag=f"s1_{mt}")
                        nc.vector.tensor_add(
                            s1n[:], s1[mt][:],
                            zh[mt // 2][:, (mt % 2) * F1 : (mt % 2 + 1) * F1],
                        )
                        s1[mt] = s1n
                        s1t.append(s1n)
                    chain1(t, s1t)
                if 1 <= s <= T:
                    fetch_rl(s - 1)

                if s >= DELAY:
                    t2 = s - DELAY
                    # z2 layout per bank: [rh(128) | x(4)] x 2 m-tiles
                    za = zp.tile([128, 2 * F1], f32, tag="z2a", name=f"z2a_{t2}")
                    zb = zp.tile([128, 2 * F1], f32, tag="z2b", name=f"z2b_{t2}")
                    zh = [za, zb]
                    mss = ms_hold.pop(t2)
                    c1s = c1_hold.pop(t2)
                    rls = rl_hold.pop(t2)
                    for nt in range(NT):
                        gt, ch0 = (nt, 0) if nt < 16 else (nt - 16, H)
                        c2 = c2p.tile([128, F2], f16, tag="c2", name="c2")
                        nc.vector.tensor_mul(
                            c2.rearrange("p (b c) -> p b c", c=H),
                            rls[gt].rearrange("p (b c) -> p b c", c=2 * H)[
                                :, :, ch0 : ch0 + H
                            ],
                            c1s[nt].rearrange("p (b k) -> p b k", k=H + 1)[
                                :, :, 1:
                            ],
                        )
                        xmv = c1s[nt].rearrange("p (b k) -> p b k", k=H + 1)[
                            :, :, 0:1
                        ]
                        for mt in range(MT):
                            off = (mt % 2) * F1
                            nc.tensor.matmul(
                                zh[mt // 2][:, off : off + F2],
                                mss[nt][:, mt * 128 : (mt + 1) * 128],
                                c2[:],
                                start=(nt == 0 and mt % 2 == 0),
                                stop=(nt == NT - 1),
                            )
                            nc.tensor.matmul(
                                zh[mt // 2][:, off + F2 : off + F2 + B],
                                mss[nt][:, mt * 128 : (mt + 1) * 128],
                                xmv,
                                start=False,
                                stop=(nt == NT - 1),
                            )
                    for mt in range(MT):
                        s2hn = accp.tile([128, F2], f32, tag=f"s2h_{mt}")
                        nc.vector.tensor_add(
                            s2hn[:], s2h[mt][:],
                            zh[mt // 2][:, (mt % 2) * F1 : (mt % 2) * F1 + F2],
                        )
                        s2h[mt] = s2hn
                        s2xn = accp.tile([128, B], f32, tag=f"s2x_{mt}")
                        nc.vector.tensor_add(
                            s2xn[:], s2x[mt][:],
                            zh[mt // 2][
                                :, (mt % 2) * F1 + F2 : (mt % 2) * F1 + F2 + B
                            ],
                        )
                        s2x[mt] = s2xn

            # ---------------- tail ----------------
            bias2f = wk.tile([128, B * H], f32, tag="bias2f", bufs=1)
            nc.sync.dma_start(
                bias2f[:], biastab[10 : 11, : B * H].broadcast_to((128, B * H))
            )
            for mt in range(MT):
                tb2 = wk.tile([128, F1], f32, tag="tbf", bufs=4)
                t2r = tb2.rearrange("p (b k) -> p b k", k=H + 1)
                nc.vector.tensor_copy(
                    t2r[:, :, 1:],
                    s2h[mt].rearrange("p (b c) -> p b c", c=H),
                )
                nc.vector.tensor_copy(
                    t2r[:, :, 0:1],
                    s2x[mt].rearrange("p (b o) -> p b o", o=1),
                )
                g2 = g1p.tile([128, 256], f32, tag="g1")
                for b in range(B):
                    tz = tpzp.tile([H + 1, 128], f32, tag="tz")
                    nc.tensor.transpose(
                        tz[:], tb2[:, b * (H + 1) : (b + 1) * (H + 1)], ident[:]
                    )
                    zbt = wk.tile([H + 1, 128], f32, tag="zbt", bufs=4)
                    nc.scalar.copy(zbt[:], tz[:])
                    nc.tensor.matmul(
                        g2[:, b * H : (b + 1) * H], zbt[:], w2s[:],
                        start=True, stop=True,
                    )
                tani = wk.tile([128, B * H], f32, tag="tani", bufs=2)
                nc.vector.scalar_tensor_tensor(
                    tani[:], g2[:, : B * H], 1.0, bias2f[:],
                    op0=Alu.mult, op1=Alu.add,
                )
                tanf = wk.tile([128, B * H], f32, tag="tanf", bufs=2)
                nc.scalar.activation(tanf[:], tani[:], Act.Tanh)
                nc.sync.dma_start(hout[mt * 128 : (mt + 1) * 128, :], tanf[:])

    nc.finalize()
    return nc


_NC_CACHE = None


def _get_nc():
    global _NC_CACHE
    if _NC_CACHE is None:
        _NC_CACHE = _build_nc()
    return _NC_CACHE


def make_in_maps(inputs, states, dtw, spec_lap, laplacian, time_delay,
                 W1, b1, W2, b2):
    inputs = np.asarray(inputs, np.float32)
    states = np.asarray(states, np.float32)
    dtw = np.asarray(dtw, np.float32)
    spec_lap = np.asarray(spec_lap, np.float32)
    laplacian = np.asarray(laplacian, np.float32)
    time_delay = np.asarray(time_delay, np.float32)
    W1 = np.asarray(W1, np.float32)
    b1 = np.asarray(b1, np.float32)
    W2 = np.asarray(W2, np.float32)
    b2 = np.asarray(b2, np.float32)

    ct_full = np.ceil(np.abs(time_delay))
    # G-space: [sigma-node, row]
    Gdtw = np.ascontiguousarray(dtw[:, PERM].T)
    Gct = np.ascontiguousarray(ct_full[:, PERM].T)
    Gsle = spec_lap[:, PERM].T.copy()
    Gsle[IPERM, np.arange(N)] += 1.0
    Glap = laplacian[:, PERM].T

    # fp16 masked matrices for every step, [T, N(sigma), N(row)]
    scratch = np.empty((N, N), np.float32)
    mst = np.empty((T, N, N), np.float16)
    for t in range(T - 1):
        np.multiply(Gdtw, (Gct > np.float32(9 - t)), out=scratch)
        scratch += Gsle
        mst[t] = scratch
    np.multiply(Gdtw, (Gct >= np.float32(1.0)), out=scratch)
    scratch += Gsle
    scratch += Glap
    mst[T - 1] = scratch

    x = inputs.transpose(1, 0, 2)               # [T, B, N]
    h = states.reshape(T, B, N, H)
    conc = np.concatenate([x[:, :, :, None], h], axis=3)  # [T,B,N,33]
    conc[9] *= 2.0 / 3.0
    concp = conc.transpose(0, 2, 1, 3)[:, PERM]  # [T, N(sigma), B, 33]
    c1all = np.ascontiguousarray(concp.reshape(T, N, F1), np.float16)

    w1hv = (0.5 * W1).astype(np.float32)
    w2hv = (0.5 * W2).astype(np.float32)
    bt = np.zeros((11, 256), np.float32)
    for t in range(T):
        bt[t] = np.tile((t + 1.0) * b1, B)
    bt[10, : B * H] = np.tile(10.0 * b2, B)

    in_maps = []
    for c in range(NC):
        rows = _rows_of(c)
        in_maps.append(
            dict(
                mstack=np.ascontiguousarray(mst[:, :, rows]),
                c1all=c1all,
                w1h=w1hv,
                w2h=w2hv,
                biastab=bt,
            )
        )
    return in_maps


def kernel(inputs, states, dtw, spec_lap, laplacian, time_delay,
           W1, b1, W2, b2):
    states = np.asarray(states, np.float32)
    in_maps = make_in_maps(
        inputs, states, dtw, spec_lap, laplacian, time_delay, W1, b1, W2, b2
    )
    nc = _get_nc()
    res = run_bass_kernel_spmd(nc, in_maps, CORES, trace=False)

    cmat = np.empty((N, B, H), np.float32)
    umat = np.empty((2048, B, 2 * H), np.float32)
    for c in range(NC):
        rows = _rows_of(c)
        cmat[rows] = np.asarray(
            res.results[c]["hout"], np.float32
        ).reshape(RPC, B, H)
        umat[256 * c : 256 * c + 256] = np.asarray(
            res.results[c]["sig9"], np.float32
        ).reshape(256, B, 2 * H)
    u = umat.transpose(1, 0, 2).reshape(B, NH)
    cfl = cmat.transpose(1, 0, 2).reshape(B, NH)
    h9 = states[T - 1]
    return (u * h9 + (1.0 - u) * cfl).astype(np.float32)
